# revision 8
# baseline (speedup 1.0000x reference)
"""Trainium2 Bass kernel for nn_ExecutionUnit_35235911696734.

Reference computation (see problem):
  prims = concat([I, softmax(primitive_scores, axis=2)])        # [16, 256, 256]
  S_t(b) = sum_p control[b, t, p] * prims[p]                    # [256, 256]
  action(b) = S_0(b) @ S_1(b) @ ... @ S_63(b)                   # chain of 64 matmuls

Strategy (data-parallel over batch, 8 chains per NeuronCore):
  * State kept transposed: Z_{t+1} = S_t^T Z_t with Z_0 = I, so every chain
    step is a plain tensor-engine matmul out = lhsT.T @ rhs with lhsT = S_t
    (stored [k, i]) and rhs = Z_t ([k, n]).  Final action = Z_64^T (host).
  * Mixing (S_t = sum_p w_p prims[p]) is done on the tensor engine with the
    primitives as the *stationary* operand, K padded to 128 by stacking 8
    different columns i of the primitives on the partition axis, and a
    block-diagonal weight matrix (built on the host, mostly zeros) as the
    moving operand.  One matmul produces S[k-half, 8 i's, 64 (b,t) pairs]
    directly in [k-partition, (i, bt)-free] layout - no transpose needed.
  * Everything on the PE runs in bf16 (fp32 4-byte moving operands stream at
    half rate on TRN2); PSUM accumulates fp32 and evacuation copies cast to
    bf16.  Positive weights mean quantization errors average out across the
    256-term contractions; measured end-to-end error is ~1e-3.
  * PSUM evacuation (the mixed S tiles and the per-step Z state) is split
    between the vector and scalar engines; two tiles are paired per PSUM
    buffer so each copy moves 1024 elements per partition.

Layouts (per core, bt = tl*8 + b within a round of T_RND=8 time steps):
  prims_w8[16h+p, o, kh, m] = prims[p, kh*128+m, 8o+h]          bf16 [128, 32, 2, 128]
  wb[16h+p, r, 64h' + bt]   = (h==h') * control[b, r*8+tl, p]   bf16 [128, 8, 512]
  ident[kappa, kh, n]       = (kh*128+kappa == n)               bf16 [128, 2, 256]
  out[b, kappa, kh, n]      = Z_64[kh*128+kappa, n]             f32  [8, 128, 2, 256]
"""

import numpy as np
import ml_dtypes

# problem constants (hardcoded - kernel.py must be self-contained)
B, T, P, D = 64, 64, 16, 256
N_CORES = 8
B_LOC = B // N_CORES          # 8 chains per core
T_RND = 8                     # time steps per mixing round
RND = T // T_RND              # 8 rounds
BT = B_LOC * T_RND            # 64 (b,t) pairs per round
OCT = 8                       # i-columns stacked per mixing weight tile
N_OCT = D // OCT              # 32 octets

_BF16 = ml_dtypes.bfloat16

_prog_cache = {}


def _build_program():
    import concourse.bass as bass
    import concourse.bacc as bacc
    import concourse.tile as tile
    import concourse.mybir as mybir

    dt = mybir.dt
    nc = bacc.Bacc()

    prims_d = nc.declare_dram_parameter(
        "prims_w8", [128, N_OCT, 2, 128], dt.bfloat16, isOutput=False)
    wb_d = nc.declare_dram_parameter(
        "wb", [128, RND, OCT * BT], dt.bfloat16, isOutput=False)
    ident_d = nc.declare_dram_parameter(
        "ident", [128, 2, 256], dt.bfloat16, isOutput=False)
    out_d = nc.declare_dram_parameter(
        "out", [B_LOC, 128, 2, 256], dt.float32, isOutput=True)

    with tile.TileContext(nc) as tc:
        with (
            tc.tile_pool(name="const", bufs=1) as cpool,
            tc.tile_pool(name="st", bufs=1) as stpool,
            tc.tile_pool(name="z", bufs=1) as zpool,
            tc.tile_pool(name="psmix", bufs=2, space=bass.MemorySpace.PSUM) as psmix,
            tc.tile_pool(name="pschain", bufs=2, space=bass.MemorySpace.PSUM) as pschain,
        ):
            prims_t = cpool.tile([128, N_OCT, 2, 128], dt.bfloat16, tag="prims")
            nc.sync.dma_start(out=prims_t[:], in_=prims_d[:])
            wb_t = cpool.tile([128, RND, OCT * BT], dt.bfloat16, tag="wb")
            nc.sync.dma_start(out=wb_t[:], in_=wb_d[:])
            ident_t = cpool.tile([128, 2, 256], dt.bfloat16, tag="ident")
            nc.sync.dma_start(out=ident_t[:], in_=ident_d[:])

            # S_T storage: [k-half partition 128, i 256, bt 64] bf16, 2 round bufs
            st = {}
            for rbuf in range(2):
                for kh in range(2):
                    st[rbuf, kh] = stpool.tile(
                        [128, D, BT], dt.bfloat16,
                        name=f"st{rbuf}{kh}", tag=f"st{rbuf}{kh}")
            # chain state, paired: z2[bp][kappa, b%2, kh, n] bf16
            z2 = {bp: zpool.tile([128, 2, 2, 256], dt.bfloat16,
                                 name=f"z{bp}", tag=f"z{bp}")
                  for bp in range(B_LOC // 2)}

            # fp32 output tiles: the last chain step lands here directly
            outt = {bp: zpool.tile([128, 2, 2, 256], dt.float32,
                                   name=f"outt{bp}", tag=f"outt{bp}")
                    for bp in range(B_LOC // 2)}

            copy_rr = [0]

            def emit_copy(dst, src):
                # split PSUM evacuation between DVE and ACT
                if copy_rr[0] % 2 == 0:
                    nc.vector.tensor_copy(dst, src)
                else:
                    nc.scalar.copy(dst, src)
                copy_rr[0] += 1

            def mix_unit(r, op, kh):
                rbuf = r % 2
                ps = psmix.tile([128, 2, OCT, BT], dt.float32,
                                name="psmix_t", tag="psmix")
                for oe in range(2):
                    o = 2 * op + oe
                    nc.tensor.matmul(
                        ps[:, oe, :, :],
                        prims_t[:, o, kh, :],      # lhsT [128, 128] bf16
                        wb_t[:, r, :],             # rhs  [128, 512] bf16
                        start=True, stop=True)
                emit_copy(
                    st[rbuf, kh][:, OCT * 2 * op:OCT * 2 * (op + 1), :],
                    ps[:])

            def emit_mix(r):
                for op in range(N_OCT // 2):
                    for kh in range(2):
                        mix_unit(r, op, kh)

            def chain_unit(r, tl, bp):
                rbuf = r % 2
                first = (r == 0 and tl == 0)
                last = (r == RND - 1 and tl == T_RND - 1)
                ps = pschain.tile([128, 2, 2, 256], dt.float32,
                                  name="pschain_t", tag="pschain")
                for be in range(2):
                    b = 2 * bp + be
                    bt = tl * B_LOC + b
                    for mh in range(2):
                        for kh in range(2):
                            rhs = (ident_t[:, kh, :] if first
                                   else z2[bp][:, be, kh, :])
                            nc.tensor.matmul(
                                ps[:, be, mh, :],
                                st[rbuf, kh][:, mh * 128:(mh + 1) * 128, bt],
                                rhs,
                                start=(kh == 0), stop=(kh == 1))
                emit_copy(outt[bp][:] if last else z2[bp][:], ps[:])

            emit_mix(0)
            emit_mix(1)
            for r in range(RND):
                # interleave the chain of round r with the mixing of round r+2
                mix_units = ([(r + 2, op, kh) for op in range(N_OCT // 2)
                              for kh in range(2)] if r + 2 < RND else [])
                slots = [(tl, bp) for tl in range(T_RND)
                         for bp in range(B_LOC // 2)]
                for i, (tl, bp) in enumerate(slots):
                    chain_unit(r, tl, bp)
                    if i < len(mix_units):
                        mix_unit(*mix_units[i])

            for bp in range(B_LOC // 2):
                for be in range(2):
                    nc.sync.dma_start(out=out_d[2 * bp + be],
                                      in_=outt[bp][:, be, :, :])

    nc.compile()
    return nc


def get_program():
    if "nc" not in _prog_cache:
        _prog_cache["nc"] = _build_program()
    return _prog_cache["nc"]


def prepare_inputs(control, primitive_scores):
    """Host-side preprocessing: softmax, layouts, sharding. Returns in_maps."""
    control = np.asarray(control, dtype=np.float32)
    scores = np.asarray(primitive_scores, dtype=np.float32)

    # fp32 softmax over axis=2, identity primitive prepended
    m = scores.max(axis=2, keepdims=True)
    e = np.exp(scores - m)
    sm = e / e.sum(axis=2, keepdims=True)
    prims = np.concatenate(
        [np.eye(D, dtype=np.float32)[None], sm.astype(np.float32)], axis=0)

    # prims_w8[16h+p, o, kh, m] = prims[p, kh*128+m, 8o+h]
    pr = prims.reshape(P, 2, 128, N_OCT, OCT)          # [p, kh, m, o, h]
    pw8 = np.ascontiguousarray(
        pr.transpose(4, 0, 3, 1, 2).reshape(128, N_OCT, 2, 128)).astype(_BF16)

    # ident[kappa, kh, n] = (kh*128 + kappa == n)
    ident = np.ascontiguousarray(
        np.eye(D, dtype=np.float32).reshape(2, 128, D).transpose(1, 0, 2)
    ).astype(_BF16)

    in_maps = []
    for d in range(N_CORES):
        w_loc = control[d * B_LOC:(d + 1) * B_LOC]     # [8, T, 16]
        # W2[p, r, tl*8 + b] = w_loc[b, r*T_RND+tl, p]
        w2 = w_loc.reshape(B_LOC, RND, T_RND, P).transpose(3, 1, 2, 0)
        w2 = w2.reshape(P, RND, BT)
        wb = np.zeros((128, RND, OCT * BT), dtype=np.float32)
        for h in range(OCT):
            wb[16 * h:16 * (h + 1), :, BT * h:BT * (h + 1)] = w2
        in_maps.append({
            "prims_w8": pw8,
            "wb": wb.astype(_BF16),
            "ident": ident,
        })
    return in_maps


def assemble_output(results):
    out = np.empty((B, D, D), dtype=np.float32)
    for d in range(N_CORES):
        arr = np.asarray(results[d]["out"], dtype=np.float32)  # [8, 128, 2, 256]
        for bl in range(B_LOC):
            zmat = arr[bl].transpose(1, 0, 2).reshape(D, D)    # Z[k, n]
            out[d * B_LOC + bl] = zmat.T                        # action = Z^T
    return out


def run(inputs, trace=False, **kwargs):
    """Run on the 8 NeuronCores. Returns (output, BassKernelResults)."""
    from concourse.bass_utils import run_bass_kernel_spmd

    nc = get_program()
    in_maps = prepare_inputs(**inputs)
    res = run_bass_kernel_spmd(
        nc, in_maps, core_ids=list(range(N_CORES)), trace=trace, **kwargs)
    return assemble_output(res.results), res


def kernel(control, primitive_scores):
    out, _ = run({"control": control, "primitive_scores": primitive_scores})
    return out


if __name__ == "__main__":
    import jax
    import reference as R

    inputs = {k: np.asarray(v) for k, v in R.setup_inputs().items()}
    out = kernel(**inputs)
    print("kernel out:", out.shape, out.dtype,
          "posinf:", np.isposinf(out).mean(), "nan:", np.isnan(out).mean())


# revision 9
# speedup vs baseline: 1.3054x; 1.3054x over previous
"""Trainium2 Bass kernel for nn_ExecutionUnit_35235911696734.

Reference computation (see problem):
  prims = concat([I, softmax(primitive_scores, axis=2)])        # [16, 256, 256]
  S_t(b) = sum_p control[b, t, p] * prims[p]                    # [256, 256]
  action(b) = S_0(b) @ S_1(b) @ ... @ S_63(b)                   # chain of 64 matmuls

Strategy (data-parallel over batch, 8 chains per NeuronCore):
  * State kept transposed: Z_{t+1} = S_t^T Z_t with Z_0 = I, so every chain
    step is a plain tensor-engine matmul out = lhsT.T @ rhs with lhsT = S_t
    (stored [k, i]) and rhs = Z_t ([k, n]).  Final action = Z_64^T (host).
  * Mixing (S_t = sum_p w_p prims[p]) is done on the tensor engine with the
    primitives as the *stationary* operand, K padded to 128 by stacking 8
    different columns i of the primitives on the partition axis, and a
    block-diagonal weight matrix (built on the host, mostly zeros) as the
    moving operand.  One matmul produces S[k-half, 8 i's, 64 (b,t) pairs]
    directly in [k-partition, (i, bt)-free] layout - no transpose needed.
  * Everything on the PE runs in bf16 (fp32 4-byte moving operands stream at
    half rate on TRN2); PSUM accumulates fp32 and evacuation copies cast to
    bf16.  Positive weights mean quantization errors average out across the
    256-term contractions; measured end-to-end error is ~1e-3.
  * PSUM evacuation (the mixed S tiles and the per-step Z state) is split
    between the vector and scalar engines; two tiles are paired per PSUM
    buffer so each copy moves 1024 elements per partition.

Layouts (per core, bt = tl*8 + b within a round of T_RND=8 time steps):
  prims_w8[16h+p, o, kh, m] = prims[p, kh*128+m, 8o+h]          bf16 [128, 32, 2, 128]
  wb[16h+p, r, 64h' + bt]   = (h==h') * control[b, r*8+tl, p]   bf16 [128, 8, 512]
  ident[kappa, kh, n]       = (kh*128+kappa == n)               bf16 [128, 2, 256]
  out[b, kappa, kh, n]      = Z_64[kh*128+kappa, n]             f32  [8, 128, 2, 256]
"""

import numpy as np
import ml_dtypes

# problem constants (hardcoded - kernel.py must be self-contained)
B, T, P, D = 64, 64, 16, 256
N_CORES = 8
B_LOC = B // N_CORES          # 8 chains per core
T_RND = 8                     # time steps per mixing round
RND = T // T_RND              # 8 rounds
BT = B_LOC * T_RND            # 64 (b,t) pairs per round
OCT = 8                       # i-columns stacked per mixing weight tile
N_OCT = D // OCT              # 32 octets

_BF16 = ml_dtypes.bfloat16

_prog_cache = {}


def _build_program():
    import concourse.bass as bass
    import concourse.bacc as bacc
    import concourse.tile as tile
    import concourse.mybir as mybir

    dt = mybir.dt
    nc = bacc.Bacc()

    prims_d = nc.declare_dram_parameter(
        "prims_w8", [128, N_OCT, 2, 128], dt.bfloat16, isOutput=False)
    wb_d = nc.declare_dram_parameter(
        "wb", [128, RND, OCT * BT], dt.bfloat16, isOutput=False)
    ident_d = nc.declare_dram_parameter(
        "ident", [128, 2, 256], dt.bfloat16, isOutput=False)
    out_d = nc.declare_dram_parameter(
        "out", [B_LOC, 128, 2, 256], dt.float32, isOutput=True)

    with tile.TileContext(nc) as tc:
        with (
            tc.tile_pool(name="const", bufs=1) as cpool,
            tc.tile_pool(name="st", bufs=1) as stpool,
            tc.tile_pool(name="z", bufs=1) as zpool,
            tc.tile_pool(name="psmix", bufs=2, space=bass.MemorySpace.PSUM) as psmix,
            tc.tile_pool(name="pschain", bufs=2, space=bass.MemorySpace.PSUM) as pschain,
        ):
            prims_t = cpool.tile([128, N_OCT, 2, 128], dt.bfloat16, tag="prims")
            nc.sync.dma_start(out=prims_t[:], in_=prims_d[:])
            wb_t = cpool.tile([128, RND, OCT * BT], dt.bfloat16, tag="wb")
            nc.sync.dma_start(out=wb_t[:], in_=wb_d[:])
            ident_t = cpool.tile([128, 2, 256], dt.bfloat16, tag="ident")
            nc.sync.dma_start(out=ident_t[:], in_=ident_d[:])

            # S_T storage: [k-half partition 128, i 256, bt 64] bf16, 2 round bufs
            st = {}
            for rbuf in range(2):
                for kh in range(2):
                    st[rbuf, kh] = stpool.tile(
                        [128, D, BT], dt.bfloat16,
                        name=f"st{rbuf}{kh}", tag=f"st{rbuf}{kh}")
            # chain state, paired: z2[bp][kappa, b%2, kh, n] bf16
            z2 = {bp: zpool.tile([128, 2, 2, 256], dt.bfloat16,
                                 name=f"z{bp}", tag=f"z{bp}")
                  for bp in range(B_LOC // 2)}

            # fp32 output tiles: the last chain step lands here directly
            outt = {bp: zpool.tile([128, 2, 2, 256], dt.float32,
                                   name=f"outt{bp}", tag=f"outt{bp}")
                    for bp in range(B_LOC // 2)}

            copy_rr = [0]

            def emit_copy(dst, src):
                # split PSUM evacuation between DVE and ACT
                if copy_rr[0] % 2 == 0:
                    nc.vector.tensor_copy(dst, src)
                else:
                    nc.scalar.copy(dst, src)
                copy_rr[0] += 1

            def mix_unit(r, op, kh):
                rbuf = r % 2
                ps = psmix.tile([128, 2, OCT, BT], dt.float32,
                                name="psmix_t", tag="psmix")
                for oe in range(2):
                    o = 2 * op + oe
                    nc.tensor.matmul(
                        ps[:, oe, :, :],
                        prims_t[:, o, kh, :],      # lhsT [128, 128] bf16
                        wb_t[:, r, :],             # rhs  [128, 512] bf16
                        start=True, stop=True)
                emit_copy(
                    st[rbuf, kh][:, OCT * 2 * op:OCT * 2 * (op + 1), :],
                    ps[:])

            def emit_mix(r):
                for op in range(N_OCT // 2):
                    for kh in range(2):
                        mix_unit(r, op, kh)

            def chain_unit(r, tl, bp):
                rbuf = r % 2
                first = (r == 0 and tl == 0)
                last = (r == RND - 1 and tl == T_RND - 1)
                ps = pschain.tile([128, 2, 2, 256], dt.float32,
                                  name="pschain_t", tag="pschain")
                for be in range(2):
                    b = 2 * bp + be
                    bt = tl * B_LOC + b
                    for mh in range(2):
                        for kh in range(2):
                            rhs = (ident_t[:, kh, :] if first
                                   else z2[bp][:, be, kh, :])
                            nc.tensor.matmul(
                                ps[:, be, mh, :],
                                st[rbuf, kh][:, mh * 128:(mh + 1) * 128, bt],
                                rhs,
                                start=(kh == 0), stop=(kh == 1))
                emit_copy(outt[bp][:] if last else z2[bp][:], ps[:])

            emit_mix(0)
            emit_mix(1)
            for r in range(RND):
                for tl in range(T_RND):
                    for bp in range(B_LOC // 2):
                        chain_unit(r, tl, bp)
                if r + 2 < RND:
                    emit_mix(r + 2)

            for bp in range(B_LOC // 2):
                for be in range(2):
                    nc.sync.dma_start(out=out_d[2 * bp + be],
                                      in_=outt[bp][:, be, :, :])

    nc.compile()
    return nc


def get_program():
    if "nc" not in _prog_cache:
        _prog_cache["nc"] = _build_program()
    return _prog_cache["nc"]


def prepare_inputs(control, primitive_scores):
    """Host-side preprocessing: softmax, layouts, sharding. Returns in_maps."""
    control = np.asarray(control, dtype=np.float32)
    scores = np.asarray(primitive_scores, dtype=np.float32)

    # fp32 softmax over axis=2, identity primitive prepended
    m = scores.max(axis=2, keepdims=True)
    e = np.exp(scores - m)
    sm = e / e.sum(axis=2, keepdims=True)
    prims = np.concatenate(
        [np.eye(D, dtype=np.float32)[None], sm.astype(np.float32)], axis=0)

    # prims_w8[16h+p, o, kh, m] = prims[p, kh*128+m, 8o+h]
    pr = prims.reshape(P, 2, 128, N_OCT, OCT)          # [p, kh, m, o, h]
    pw8 = np.ascontiguousarray(
        pr.transpose(4, 0, 3, 1, 2).reshape(128, N_OCT, 2, 128)).astype(_BF16)

    # ident[kappa, kh, n] = (kh*128 + kappa == n)
    ident = np.ascontiguousarray(
        np.eye(D, dtype=np.float32).reshape(2, 128, D).transpose(1, 0, 2)
    ).astype(_BF16)

    in_maps = []
    for d in range(N_CORES):
        w_loc = control[d * B_LOC:(d + 1) * B_LOC]     # [8, T, 16]
        # W2[p, r, tl*8 + b] = w_loc[b, r*T_RND+tl, p]
        w2 = w_loc.reshape(B_LOC, RND, T_RND, P).transpose(3, 1, 2, 0)
        w2 = w2.reshape(P, RND, BT)
        wb = np.zeros((128, RND, OCT * BT), dtype=np.float32)
        for h in range(OCT):
            wb[16 * h:16 * (h + 1), :, BT * h:BT * (h + 1)] = w2
        in_maps.append({
            "prims_w8": pw8,
            "wb": wb.astype(_BF16),
            "ident": ident,
        })
    return in_maps


def assemble_output(results):
    out = np.empty((B, D, D), dtype=np.float32)
    for d in range(N_CORES):
        arr = np.asarray(results[d]["out"], dtype=np.float32)  # [8, 128, 2, 256]
        for bl in range(B_LOC):
            zmat = arr[bl].transpose(1, 0, 2).reshape(D, D)    # Z[k, n]
            out[d * B_LOC + bl] = zmat.T                        # action = Z^T
    return out


def run(inputs, trace=False, **kwargs):
    """Run on the 8 NeuronCores. Returns (output, BassKernelResults)."""
    from concourse.bass_utils import run_bass_kernel_spmd

    nc = get_program()
    in_maps = prepare_inputs(**inputs)
    res = run_bass_kernel_spmd(
        nc, in_maps, core_ids=list(range(N_CORES)), trace=trace, **kwargs)
    return assemble_output(res.results), res


def kernel(control, primitive_scores):
    out, _ = run({"control": control, "primitive_scores": primitive_scores})
    return out


if __name__ == "__main__":
    import jax
    import reference as R

    inputs = {k: np.asarray(v) for k, v in R.setup_inputs().items()}
    out = kernel(**inputs)
    print("kernel out:", out.shape, out.dtype,
          "posinf:", np.isposinf(out).mean(), "nan:", np.isnan(out).mean())


# revision 10
# speedup vs baseline: 1.4269x; 1.0931x over previous
"""Trainium2 Bass kernel for nn_ExecutionUnit_35235911696734.

Reference computation (see problem):
  prims = concat([I, softmax(primitive_scores, axis=2)])        # [16, 256, 256]
  S_t(b) = sum_p control[b, t, p] * prims[p]                    # [256, 256]
  action(b) = S_0(b) @ S_1(b) @ ... @ S_63(b)                   # chain of 64 matmuls

Strategy (data-parallel over batch, 8 chains per NeuronCore):
  * State kept transposed: Z_{t+1} = S_t^T Z_t with Z_0 = I, so every chain
    step is a plain tensor-engine matmul out = lhsT.T @ rhs with lhsT = S_t
    (stored [k, i]) and rhs = Z_t ([k, n]).  Final action = Z_64^T (host).
  * Mixing (S_t = sum_p w_p prims[p]) is done on the tensor engine with the
    primitives as the *stationary* operand, K padded to 128 by stacking 8
    different columns i of the primitives on the partition axis, and a
    block-diagonal weight matrix (built on the host, mostly zeros) as the
    moving operand.  One matmul produces S[k-half, 8 i's, 64 (b,t) pairs]
    directly in [k-partition, (i, bt)-free] layout - no transpose needed.
  * Everything on the PE runs in bf16 (fp32 4-byte moving operands stream at
    half rate on TRN2); PSUM accumulates fp32 and evacuation copies cast to
    bf16.  Positive weights mean quantization errors average out across the
    256-term contractions; measured end-to-end error is ~1e-3.
  * PSUM evacuation (the mixed S tiles and the per-step Z state) is split
    between the vector and scalar engines; two tiles are paired per PSUM
    buffer so each copy moves 1024 elements per partition.

Layouts (per core, bt = tl*8 + b within a round of T_RND=8 time steps):
  prims_w8[16h+p, o, kh, m] = prims[p, kh*128+m, 8o+h]          bf16 [128, 32, 2, 128]
  wb[16h+p, r, 64h' + bt]   = (h==h') * control[b, r*8+tl, p]   bf16 [128, 8, 512]
  ident[kappa, kh, n]       = (kh*128+kappa == n)               bf16 [128, 2, 256]
  out[b, kappa, kh, n]      = Z_64[kh*128+kappa, n]             f32  [8, 128, 2, 256]
"""

import numpy as np
import ml_dtypes

# problem constants (hardcoded - kernel.py must be self-contained)
B, T, P, D = 64, 64, 16, 256
N_CORES = 8
B_LOC = B // N_CORES          # 8 chains per core
T_RND = 4                     # time steps per mixing round
RND = T // T_RND              # 16 rounds
BT = B_LOC * T_RND            # 32 (b,t) pairs per round
ST_BUFS = 3                   # S_T round buffers (2-round mixing lookahead)
OCT = 8                       # i-columns stacked per mixing weight tile
N_OCT = D // OCT              # 32 octets

_BF16 = ml_dtypes.bfloat16

_prog_cache = {}


def _build_program():
    import concourse.bass as bass
    import concourse.bacc as bacc
    import concourse.tile as tile
    import concourse.mybir as mybir

    dt = mybir.dt
    nc = bacc.Bacc()

    prims_d = nc.declare_dram_parameter(
        "prims_w8", [128, N_OCT, 2, 128], dt.bfloat16, isOutput=False)
    wb_d = nc.declare_dram_parameter(
        "wb", [128, RND, OCT * BT], dt.bfloat16, isOutput=False)
    ident_d = nc.declare_dram_parameter(
        "ident", [128, 2, 256], dt.bfloat16, isOutput=False)
    out_d = nc.declare_dram_parameter(
        "out", [B_LOC, 128, 2, 256], dt.float32, isOutput=True)

    with tile.TileContext(nc) as tc:
        with (
            tc.tile_pool(name="const", bufs=1) as cpool,
            tc.tile_pool(name="st", bufs=1) as stpool,
            tc.tile_pool(name="z", bufs=1) as zpool,
            tc.tile_pool(name="psmix", bufs=2, space=bass.MemorySpace.PSUM) as psmix,
            tc.tile_pool(name="pschain", bufs=2, space=bass.MemorySpace.PSUM) as pschain,
        ):
            prims_t = cpool.tile([128, N_OCT, 2, 128], dt.bfloat16, tag="prims")
            nc.sync.dma_start(out=prims_t[:], in_=prims_d[:])
            wb_t = cpool.tile([128, RND, OCT * BT], dt.bfloat16, tag="wb")
            nc.sync.dma_start(out=wb_t[:], in_=wb_d[:])
            ident_t = cpool.tile([128, 2, 256], dt.bfloat16, tag="ident")
            nc.sync.dma_start(out=ident_t[:], in_=ident_d[:])

            # S_T storage: [k-half partition 128, i 256, bt 32] bf16, 3 round bufs
            st = {}
            for rbuf in range(ST_BUFS):
                for kh in range(2):
                    st[rbuf, kh] = stpool.tile(
                        [128, D, BT], dt.bfloat16,
                        name=f"st{rbuf}{kh}", tag=f"st{rbuf}{kh}")
            # chain state, paired: z2[bp][kappa, b%2, kh, n] bf16
            z2 = {bp: zpool.tile([128, 2, 2, 256], dt.bfloat16,
                                 name=f"z{bp}", tag=f"z{bp}")
                  for bp in range(B_LOC // 2)}

            # fp32 output tiles: the last chain step lands here directly
            outt = {bp: zpool.tile([128, 2, 2, 256], dt.float32,
                                   name=f"outt{bp}", tag=f"outt{bp}")
                    for bp in range(B_LOC // 2)}

            copy_rr = [0]

            def emit_copy(dst, src):
                # split PSUM evacuation between DVE and ACT
                if copy_rr[0] % 2 == 0:
                    nc.vector.tensor_copy(dst, src)
                else:
                    nc.scalar.copy(dst, src)
                copy_rr[0] += 1

            def mix_unit(r, oq, kh):
                # one PSUM tile covers 4 octets (FD=1024)
                rbuf = r % ST_BUFS
                ps = psmix.tile([128, 4, OCT, BT], dt.float32,
                                name="psmix_t", tag="psmix")
                for oe in range(4):
                    o = 4 * oq + oe
                    nc.tensor.matmul(
                        ps[:, oe, :, :],
                        prims_t[:, o, kh, :],      # lhsT [128, 128] bf16
                        wb_t[:, r, :],             # rhs  [128, 256] bf16
                        start=True, stop=True)
                emit_copy(
                    st[rbuf, kh][:, OCT * 4 * oq:OCT * 4 * (oq + 1), :],
                    ps[:])

            def emit_mix(r):
                for oq in range(N_OCT // 4):
                    for kh in range(2):
                        mix_unit(r, oq, kh)

            def chain_unit(r, tl, bp):
                rbuf = r % ST_BUFS
                first = (r == 0 and tl == 0)
                last = (r == RND - 1 and tl == T_RND - 1)
                ps = pschain.tile([128, 2, 2, 256], dt.float32,
                                  name="pschain_t", tag="pschain")
                for be in range(2):
                    b = 2 * bp + be
                    bt = tl * B_LOC + b
                    for mh in range(2):
                        for kh in range(2):
                            rhs = (ident_t[:, kh, :] if first
                                   else z2[bp][:, be, kh, :])
                            nc.tensor.matmul(
                                ps[:, be, mh, :],
                                st[rbuf, kh][:, mh * 128:(mh + 1) * 128, bt],
                                rhs,
                                start=(kh == 0), stop=(kh == 1))
                emit_copy(outt[bp][:] if last else z2[bp][:], ps[:])

            emit_mix(0)
            emit_mix(1)
            for r in range(RND):
                for tl in range(T_RND):
                    for bp in range(B_LOC // 2):
                        chain_unit(r, tl, bp)
                if r + 2 < RND:
                    emit_mix(r + 2)

            for bp in range(B_LOC // 2):
                for be in range(2):
                    nc.sync.dma_start(out=out_d[2 * bp + be],
                                      in_=outt[bp][:, be, :, :])

    nc.compile()
    return nc


def get_program():
    if "nc" not in _prog_cache:
        _prog_cache["nc"] = _build_program()
    return _prog_cache["nc"]


def prepare_inputs(control, primitive_scores):
    """Host-side preprocessing: softmax, layouts, sharding. Returns in_maps."""
    control = np.asarray(control, dtype=np.float32)
    scores = np.asarray(primitive_scores, dtype=np.float32)

    # fp32 softmax over axis=2, identity primitive prepended
    m = scores.max(axis=2, keepdims=True)
    e = np.exp(scores - m)
    sm = e / e.sum(axis=2, keepdims=True)
    prims = np.concatenate(
        [np.eye(D, dtype=np.float32)[None], sm.astype(np.float32)], axis=0)

    # prims_w8[16h+p, o, kh, m] = prims[p, kh*128+m, 8o+h]
    pr = prims.reshape(P, 2, 128, N_OCT, OCT)          # [p, kh, m, o, h]
    pw8 = np.ascontiguousarray(
        pr.transpose(4, 0, 3, 1, 2).reshape(128, N_OCT, 2, 128)).astype(_BF16)

    # ident[kappa, kh, n] = (kh*128 + kappa == n)
    ident = np.ascontiguousarray(
        np.eye(D, dtype=np.float32).reshape(2, 128, D).transpose(1, 0, 2)
    ).astype(_BF16)

    in_maps = []
    for d in range(N_CORES):
        w_loc = control[d * B_LOC:(d + 1) * B_LOC]     # [8, T, 16]
        # W2[p, r, tl*8 + b] = w_loc[b, r*T_RND+tl, p]
        w2 = w_loc.reshape(B_LOC, RND, T_RND, P).transpose(3, 1, 2, 0)
        w2 = w2.reshape(P, RND, BT)
        wb = np.zeros((128, RND, OCT * BT), dtype=np.float32)
        for h in range(OCT):
            wb[16 * h:16 * (h + 1), :, BT * h:BT * (h + 1)] = w2
        in_maps.append({
            "prims_w8": pw8,
            "wb": wb.astype(_BF16),
            "ident": ident,
        })
    return in_maps


def assemble_output(results):
    out = np.empty((B, D, D), dtype=np.float32)
    for d in range(N_CORES):
        arr = np.asarray(results[d]["out"], dtype=np.float32)  # [8, 128, 2, 256]
        for bl in range(B_LOC):
            zmat = arr[bl].transpose(1, 0, 2).reshape(D, D)    # Z[k, n]
            out[d * B_LOC + bl] = zmat.T                        # action = Z^T
    return out


def run(inputs, trace=False, **kwargs):
    """Run on the 8 NeuronCores. Returns (output, BassKernelResults)."""
    from concourse.bass_utils import run_bass_kernel_spmd

    nc = get_program()
    in_maps = prepare_inputs(**inputs)
    res = run_bass_kernel_spmd(
        nc, in_maps, core_ids=list(range(N_CORES)), trace=trace, **kwargs)
    return assemble_output(res.results), res


def kernel(control, primitive_scores):
    out, _ = run({"control": control, "primitive_scores": primitive_scores})
    return out


if __name__ == "__main__":
    import jax
    import reference as R

    inputs = {k: np.asarray(v) for k, v in R.setup_inputs().items()}
    out = kernel(**inputs)
    print("kernel out:", out.shape, out.dtype,
          "posinf:", np.isposinf(out).mean(), "nan:", np.isnan(out).mean())
